# revision 26
# baseline (speedup 1.0000x reference)
"""Mixtral-style MoE (top-2 of 8 experts) on 8 TRN2 NeuronCores.

Strategy (expert-parallel, matching TENSOR_EXPERT_PARALLEL):
  - Host: router (logits -> softmax -> top-2 -> normalized weights), then
    shard: core e receives the tokens routed to expert e (gathered and
    pre-transposed to [H, C]) plus expert e's w1/w3/w2 (bf16, pre-packed
    into PE-friendly [128 x free] tiles).
  - Device (SPMD, identical program on 8 cores): h1T = w1 @ xeT,
    h3T = w3 @ xeT, gT = silu(h1T) * h3T (bf16), outT = gT.T @ w2T,
    scaled per-token by the routing weight.  Pure GEMM pipeline; all
    DMAs are fully linear.
  - Host: scatter-add each core's [count_e, H] contribution into the
    [T, H] output (each token appears in exactly TOP_K=2 expert lists).

Compute is done in bf16 (fp32 accumulation in PSUM), which keeps the
TensorEngine at its 78.6 TF/s peak; sparse routing means each core does
~C=1152 token-columns instead of all 4096 (4x fewer FLOPs than dense).
"""

import numpy as np
import ml_dtypes

B, S, H, F, E, TOP_K = 2, 2048, 1024, 3584, 8, 2
N_CORES = 8
P = 128
HK = H // P   # 8 contraction chunks for up-proj
FP = F // P   # 28 partition chunks of the FFN dim

BF16 = ml_dtypes.bfloat16

_BUILD_CACHE = {}
LAST_EXEC_TIME_NS = None


def _ensure_axon_hooks_stub():
    """bass_utils imports antenv.axon_hooks when BASS_TRACE is set; the
    agent image lacks it.  Register a None-hook stub so a stray
    BASS_TRACE env var degrades to an untraced run instead of crashing.
    """
    import sys, types

    try:
        import antenv.axon_hooks  # noqa: F401
        return
    except ImportError:
        pass
    mod = types.ModuleType("antenv.axon_hooks")
    mod._hook = None
    mod.set_axon_ntff_profile_hook = lambda h: setattr(mod, "_hook", h)
    mod.get_axon_ntff_profile_hook = lambda: mod._hook
    sys.modules["antenv.axon_hooks"] = mod
    try:
        import antenv

        antenv.axon_hooks = mod
    except ImportError:
        pass


def _chunks(total, maxc):
    """Split `total` into equal-ish chunks <= maxc (PSUM free-dim cap)."""
    n = -(-total // maxc)
    base, rem = divmod(total, n)
    sizes = [base + (1 if i < rem else 0) for i in range(n)]
    out, off = [], 0
    for c in sizes:
        out.append((off, c))
        off += c
    return out


def _build(C):
    """Build + compile the SPMD Bass program for token capacity C."""
    import concourse.bacc as bacc
    import concourse.mybir as mybir
    from concourse.tile import TileContext

    bf = mybir.dt.bfloat16
    f32 = mybir.dt.float32
    CK = -(-C // P)  # token-partition chunks in phase B (last may be partial)

    nc = bacc.Bacc("TRN2", target_bir_lowering=False, debug=False,
                   num_devices=N_CORES)
    xe = nc.dram_tensor("xe", [HK, P, C], bf, kind="ExternalInput")
    w1p = nc.dram_tensor("w1p", [FP, P, H], bf, kind="ExternalInput")
    w3p = nc.dram_tensor("w3p", [FP, P, H], bf, kind="ExternalInput")
    w2p = nc.dram_tensor("w2p", [FP, P, H], bf, kind="ExternalInput")
    cv = nc.dram_tensor("cv", [C, 1], f32, kind="ExternalInput")
    out = nc.dram_tensor("out", [C, H], f32, kind="ExternalOutput")

    cn_chunks = _chunks(C, 512)
    silu = mybir.ActivationFunctionType.Silu
    copy = mybir.ActivationFunctionType.Copy

    with TileContext(nc) as tc:
        with (
            tc.tile_pool(name="persist", bufs=1) as persist,
            tc.tile_pool(name="wload", bufs=4) as wload,
            tc.tile_pool(name="gpool", bufs=1) as gpool,
            tc.tile_pool(name="evac", bufs=3) as evac,
            tc.tile_pool(name="ost", bufs=3) as ost,
        ):
            # Startup-critical DMA order: fp0 panels, first activation
            # chunks, fp1 panels, rest of the activations.  fp0's hk-outer
            # matmul chain consumes xe[hk] at ~1us/chunk, so fp1's panels
            # slot in without delaying it.
            # Startup: spread descriptor issue across engines (DIRECT2D is
            # ~0.6us/issue per sequencer; serial issue would gate the PE).
            w1t0 = wload.tile([P, H], bf, tag="w1")
            nc.sync.dma_start(out=w1t0[:], in_=w1p[0])
            w3t0 = wload.tile([P, H], bf, tag="w3")
            nc.sync.dma_start(out=w3t0[:], in_=w3p[0])
            w1t1 = wload.tile([P, H], bf, tag="w1")
            nc.sync.dma_start(out=w1t1[:], in_=w1p[1])
            w3t1 = wload.tile([P, H], bf, tag="w3")
            nc.sync.dma_start(out=w3t1[:], in_=w3p[1])

            # Scalar is also HWDGE: split descriptor issue across the two
            # sequencers so the ~0.6us/DIRECT2D issue cost parallelizes.
            xet = [persist.tile([P, C], bf, tag=f"xe{hk}", name=f"xe{hk}")
                   for hk in range(HK)]
            for hk in range(HK):
                e = nc.scalar if hk % 2 == 0 else nc.sync
                e.dma_start(out=xet[hk][:], in_=xe[hk])
            gt = [gpool.tile([P, C], bf, tag=f"g{fp}", name=f"g{fp}")
                  for fp in range(FP)]

            # Phase A: h1T/h3T = w1/w3 @ xeT per 128-row chunk of F,
            # fused SwiGLU into gT (bf16).
            with tc.tile_pool(name="psA", bufs=4, space="PSUM") as psA:
                for fp in range(FP):
                    if fp < 2:
                        # hk-outer: each matmul chain consumes xe[hk] as it
                        # lands instead of stalling on the whole activation
                        # load before the first instruction.  Two chains
                        # (~15us PE) cover the ~9us startup DMA window.
                        w1t, w3t = (w1t0, w3t0) if fp == 0 else (w1t1, w3t1)
                        pss = {}
                        for mat in (1, 3):
                            for ci in range(len(cn_chunks)):
                                pss[(mat, ci)] = psA.tile(
                                    [P, 512], f32, tag=f"ps{mat}",
                                    name=f"ps{mat}_c{ci}_f{fp}",
                                )
                        for hk in range(HK):
                            for mat, wt in ((1, w1t), (3, w3t)):
                                for ci, (coff, csz) in enumerate(cn_chunks):
                                    nc.tensor.matmul(
                                        pss[(mat, ci)][:, :csz],
                                        wt[:, hk * P:(hk + 1) * P],
                                        xet[hk][:, coff:coff + csz],
                                        start=(hk == 0), stop=(hk == HK - 1),
                                    )
                        for ci, (coff, csz) in enumerate(cn_chunks):
                            sil = evac.tile([P, 512], f32, tag="sil",
                                            name=f"sil_f{fp}_{ci}")
                            nc.scalar.activation(
                                sil[:, :csz], pss[(1, ci)][:, :csz], silu)
                            nc.vector.tensor_mul(
                                gt[fp][:, coff:coff + csz], sil[:, :csz],
                                pss[(3, ci)][:, :csz],
                            )
                        continue
                    else:
                        w1t = wload.tile([P, H], bf, tag="w1")
                        nc.sync.dma_start(out=w1t[:], in_=w1p[fp])
                        w3t = wload.tile([P, H], bf, tag="w3")
                        nc.sync.dma_start(out=w3t[:], in_=w3p[fp])
                    for (coff, csz) in cn_chunks:
                        ps1 = psA.tile([P, 512], f32, tag="ps1")
                        ps3 = psA.tile([P, 512], f32, tag="ps3")
                        for hk in range(HK):
                            nc.tensor.matmul(
                                ps1[:, :csz],
                                w1t[:, hk * P:(hk + 1) * P],
                                xet[hk][:, coff:coff + csz],
                                start=(hk == 0), stop=(hk == HK - 1),
                            )
                        for hk in range(HK):
                            nc.tensor.matmul(
                                ps3[:, :csz],
                                w3t[:, hk * P:(hk + 1) * P],
                                xet[hk][:, coff:coff + csz],
                                start=(hk == 0), stop=(hk == HK - 1),
                            )
                        sil = evac.tile([P, 512], f32, tag="sil")
                        nc.scalar.activation(sil[:, :csz], ps1[:, :csz], silu)
                        nc.vector.tensor_mul(
                            gt[fp][:, coff:coff + csz], sil[:, :csz], ps3[:, :csz]
                        )

            # w2 / routing-weight loads are only needed in phase B; emit
            # them after phase A so the DMA queues serve phase A first.
            cvt = []
            for ck in range(CK):
                m = min(P, C - ck * P)
                t = persist.tile([P, 1], f32, tag=f"cv{ck}", name=f"cv{ck}")
                nc.sync.dma_start(out=t[:m, :], in_=cv[ck * P:ck * P + m, :])
                cvt.append(t)
            w2t = []
            for fp in range(FP):
                t = persist.tile([P, H], bf, tag=f"w2_{fp}", name=f"w2_{fp}")
                nc.sync.dma_start(out=t[:], in_=w2p[fp])
                w2t.append(t)

            # Phase B: outT chunk [128 tokens, 1024] = sum_f gT.T @ w2T,
            # scaled by the per-token routing weight on eviction.
            with tc.tile_pool(name="psB", bufs=2, space="PSUM") as psB:
                for ck in range(CK):
                    m = min(P, C - ck * P)
                    pb0 = psB.tile([P, 512], f32, tag="pb0")
                    pb1 = psB.tile([P, 512], f32, tag="pb1")
                    for fp in range(FP):
                        lhs = gt[fp][:, ck * P:ck * P + m]
                        nc.tensor.matmul(pb0[:m, :], lhs, w2t[fp][:, 0:512],
                                         start=(fp == 0), stop=(fp == FP - 1))
                        nc.tensor.matmul(pb1[:m, :], lhs, w2t[fp][:, 512:1024],
                                         start=(fp == 0), stop=(fp == FP - 1))
                    o = ost.tile([P, H], f32, tag="o")
                    nc.scalar.activation(o[:m, 0:512], pb0[:m, :], copy,
                                         scale=cvt[ck][:m, :])
                    nc.scalar.activation(o[:m, 512:1024], pb1[:m, :], copy,
                                         scale=cvt[ck][:m, :])
                    nc.sync.dma_start(out=out[ck * P:ck * P + m, :], in_=o[:m, :])

    nc.compile()
    return nc


def kernel(hidden_states, gate_w, w1, w2, w3, _trace=False):
    global LAST_EXEC_TIME_NS
    _ensure_axon_hooks_stub()
    from concourse.bass_utils import run_bass_kernel_spmd

    x = np.asarray(hidden_states, dtype=np.float32).reshape(-1, H)
    gate_w = np.asarray(gate_w, dtype=np.float32)
    w1 = np.asarray(w1, dtype=np.float32)
    w2 = np.asarray(w2, dtype=np.float32)
    w3 = np.asarray(w3, dtype=np.float32)
    T = x.shape[0]

    # Router (f32, same math as the module): softmax over experts, top-2,
    # renormalized weights.
    logits = x @ gate_w.T
    p = np.exp(logits - logits.max(-1, keepdims=True))
    p /= p.sum(-1, keepdims=True)
    sel = np.argpartition(-p, TOP_K - 1, axis=-1)[:, :TOP_K]
    rw = np.take_along_axis(p, sel, axis=-1)
    rw = rw / rw.sum(-1, keepdims=True)

    idx_e, cv_e = [], []
    for e in range(E):
        hit = sel == e                      # [T, K]
        idx = np.nonzero(hit.any(axis=1))[0]
        w = np.where(hit[idx, 0], rw[idx, 0], rw[idx, 1])
        idx_e.append(idx)
        cv_e.append(w.astype(np.float32))

    cmax = max(len(i) for i in idx_e)
    C = max(cmax, 512)
    # SBUF budget (xe + gT residents) caps C; actual data peaks ~1071.
    assert C <= 1408, f"capacity {C} exceeds SBUF plan"

    if C not in _BUILD_CACHE:
        _BUILD_CACHE[C] = _build(C)
    nc = _BUILD_CACHE[C]

    x_bf = x.astype(BF16)
    in_maps = []
    for e in range(E):
        n = len(idx_e[e])
        xeT = np.zeros((H, C), dtype=BF16)
        xeT[:, :n] = x_bf[idx_e[e]].T
        w1pk = np.ascontiguousarray(
            w1[e].astype(BF16).reshape(FP, P, HK, P).transpose(0, 3, 2, 1)
        ).reshape(FP, P, H)
        w3pk = np.ascontiguousarray(
            w3[e].astype(BF16).reshape(FP, P, HK, P).transpose(0, 3, 2, 1)
        ).reshape(FP, P, H)
        w2pk = np.ascontiguousarray(w2[e].T.astype(BF16)).reshape(FP, P, H)
        cvp = np.zeros((C, 1), dtype=np.float32)
        cvp[:n, 0] = cv_e[e]
        in_maps.append({
            "xe": np.ascontiguousarray(xeT.reshape(HK, P, C)),
            "w1p": w1pk,
            "w3p": w3pk,
            "w2p": w2pk,
            "cv": cvp,
        })

    res = run_bass_kernel_spmd(
        nc, in_maps, core_ids=list(range(N_CORES)), trace=_trace
    )
    LAST_EXEC_TIME_NS = res.exec_time_ns

    out = np.zeros((T, H), dtype=np.float32)
    for e in range(E):
        n = len(idx_e[e])
        oe = res.results[e]["out"].reshape(C, H)[:n]
        out[idx_e[e]] += oe
    return out.reshape(B, S, H)


# revision 27
# speedup vs baseline: 1.0005x; 1.0005x over previous
"""Mixtral-style MoE (top-2 of 8 experts) on 8 TRN2 NeuronCores.

Strategy (expert-parallel, matching TENSOR_EXPERT_PARALLEL):
  - Host: router (logits -> softmax -> top-2 -> normalized weights), then
    shard: core e receives the tokens routed to expert e (gathered and
    pre-transposed to [H, C]) plus expert e's w1/w3/w2 (bf16, pre-packed
    into PE-friendly [128 x free] tiles).
  - Device (SPMD, identical program on 8 cores): h1T = w1 @ xeT,
    h3T = w3 @ xeT, gT = silu(h1T) * h3T (bf16), outT = gT.T @ w2T,
    scaled per-token by the routing weight.  Pure GEMM pipeline; all
    DMAs are fully linear.
  - Host: scatter-add each core's [count_e, H] contribution into the
    [T, H] output (each token appears in exactly TOP_K=2 expert lists).

Compute is done in bf16 (fp32 accumulation in PSUM), which keeps the
TensorEngine at its 78.6 TF/s peak; sparse routing means each core does
~C=1152 token-columns instead of all 4096 (4x fewer FLOPs than dense).
"""

import numpy as np
import ml_dtypes

B, S, H, F, E, TOP_K = 2, 2048, 1024, 3584, 8, 2
N_CORES = 8
P = 128
HK = H // P   # 8 contraction chunks for up-proj
FP = F // P   # 28 partition chunks of the FFN dim

BF16 = ml_dtypes.bfloat16

_BUILD_CACHE = {}
LAST_EXEC_TIME_NS = None


def _ensure_axon_hooks_stub():
    """bass_utils imports antenv.axon_hooks when BASS_TRACE is set; the
    agent image lacks it.  Register a None-hook stub so a stray
    BASS_TRACE env var degrades to an untraced run instead of crashing.
    """
    import sys, types

    try:
        import antenv.axon_hooks  # noqa: F401
        return
    except ImportError:
        pass
    mod = types.ModuleType("antenv.axon_hooks")
    mod._hook = None
    mod.set_axon_ntff_profile_hook = lambda h: setattr(mod, "_hook", h)
    mod.get_axon_ntff_profile_hook = lambda: mod._hook
    sys.modules["antenv.axon_hooks"] = mod
    try:
        import antenv

        antenv.axon_hooks = mod
    except ImportError:
        pass


def _chunks(total, maxc):
    """Split `total` into equal-ish chunks <= maxc (PSUM free-dim cap)."""
    n = -(-total // maxc)
    base, rem = divmod(total, n)
    sizes = [base + (1 if i < rem else 0) for i in range(n)]
    out, off = [], 0
    for c in sizes:
        out.append((off, c))
        off += c
    return out


def _build(C):
    """Build + compile the SPMD Bass program for token capacity C."""
    import concourse.bacc as bacc
    import concourse.mybir as mybir
    from concourse.tile import TileContext

    bf = mybir.dt.bfloat16
    f32 = mybir.dt.float32
    CK = -(-C // P)  # token-partition chunks in phase B (last may be partial)

    nc = bacc.Bacc("TRN2", target_bir_lowering=False, debug=False,
                   num_devices=N_CORES)
    xe = nc.dram_tensor("xe", [HK, P, C], bf, kind="ExternalInput")
    w1p = nc.dram_tensor("w1p", [FP, P, H], bf, kind="ExternalInput")
    w3p = nc.dram_tensor("w3p", [FP, P, H], bf, kind="ExternalInput")
    w2p = nc.dram_tensor("w2p", [FP, P, H], bf, kind="ExternalInput")
    cv = nc.dram_tensor("cv", [C, 1], f32, kind="ExternalInput")
    out = nc.dram_tensor("out", [C, H], f32, kind="ExternalOutput")

    cn_chunks = _chunks(C, 512)
    silu = mybir.ActivationFunctionType.Silu
    copy = mybir.ActivationFunctionType.Copy

    with TileContext(nc) as tc:
        with (
            tc.tile_pool(name="persist", bufs=1) as persist,
            tc.tile_pool(name="wload", bufs=4) as wload,
            tc.tile_pool(name="gpool", bufs=1) as gpool,
            tc.tile_pool(name="evac", bufs=3) as evac,
            tc.tile_pool(name="ost", bufs=3) as ost,
        ):
            # Startup-critical DMA order: fp0 panels, first activation
            # chunks, fp1 panels, rest of the activations.  fp0's hk-outer
            # matmul chain consumes xe[hk] at ~1us/chunk, so fp1's panels
            # slot in without delaying it.
            # Startup: spread descriptor issue across engines (DIRECT2D is
            # ~0.6us/issue per sequencer; serial issue would gate the PE).
            w1t0 = wload.tile([P, H], bf, tag="w1")
            nc.sync.dma_start(out=w1t0[:], in_=w1p[0])
            w3t0 = wload.tile([P, H], bf, tag="w3")
            nc.sync.dma_start(out=w3t0[:], in_=w3p[0])
            w1t1 = wload.tile([P, H], bf, tag="w1")
            nc.sync.dma_start(out=w1t1[:], in_=w1p[1])
            w3t1 = wload.tile([P, H], bf, tag="w3")
            nc.sync.dma_start(out=w3t1[:], in_=w3p[1])

            xet = [persist.tile([P, C], bf, tag=f"xe{hk}", name=f"xe{hk}")
                   for hk in range(HK)]
            for hk in range(HK):
                nc.sync.dma_start(out=xet[hk][:], in_=xe[hk])
            gt = [gpool.tile([P, C], bf, tag=f"g{fp}", name=f"g{fp}")
                  for fp in range(FP)]

            # Phase A: h1T/h3T = w1/w3 @ xeT per 128-row chunk of F,
            # fused SwiGLU into gT (bf16).
            with tc.tile_pool(name="psA", bufs=4, space="PSUM") as psA:
                for fp in range(FP):
                    if fp < 2:
                        # hk-outer: each matmul chain consumes xe[hk] as it
                        # lands instead of stalling on the whole activation
                        # load before the first instruction.  Two chains
                        # (~15us PE) cover the ~9us startup DMA window.
                        w1t, w3t = (w1t0, w3t0) if fp == 0 else (w1t1, w3t1)
                        pss = {}
                        for mat in (1, 3):
                            for ci in range(len(cn_chunks)):
                                pss[(mat, ci)] = psA.tile(
                                    [P, 512], f32, tag=f"ps{mat}",
                                    name=f"ps{mat}_c{ci}_f{fp}",
                                )
                        for hk in range(HK):
                            for mat, wt in ((1, w1t), (3, w3t)):
                                for ci, (coff, csz) in enumerate(cn_chunks):
                                    nc.tensor.matmul(
                                        pss[(mat, ci)][:, :csz],
                                        wt[:, hk * P:(hk + 1) * P],
                                        xet[hk][:, coff:coff + csz],
                                        start=(hk == 0), stop=(hk == HK - 1),
                                    )
                        for ci, (coff, csz) in enumerate(cn_chunks):
                            sil = evac.tile([P, 512], f32, tag="sil",
                                            name=f"sil_f{fp}_{ci}")
                            nc.scalar.activation(
                                sil[:, :csz], pss[(1, ci)][:, :csz], silu)
                            nc.vector.tensor_mul(
                                gt[fp][:, coff:coff + csz], sil[:, :csz],
                                pss[(3, ci)][:, :csz],
                            )
                        continue
                    else:
                        w1t = wload.tile([P, H], bf, tag="w1")
                        nc.sync.dma_start(out=w1t[:], in_=w1p[fp])
                        w3t = wload.tile([P, H], bf, tag="w3")
                        nc.sync.dma_start(out=w3t[:], in_=w3p[fp])
                    for (coff, csz) in cn_chunks:
                        ps1 = psA.tile([P, 512], f32, tag="ps1")
                        ps3 = psA.tile([P, 512], f32, tag="ps3")
                        for hk in range(HK):
                            nc.tensor.matmul(
                                ps1[:, :csz],
                                w1t[:, hk * P:(hk + 1) * P],
                                xet[hk][:, coff:coff + csz],
                                start=(hk == 0), stop=(hk == HK - 1),
                            )
                        for hk in range(HK):
                            nc.tensor.matmul(
                                ps3[:, :csz],
                                w3t[:, hk * P:(hk + 1) * P],
                                xet[hk][:, coff:coff + csz],
                                start=(hk == 0), stop=(hk == HK - 1),
                            )
                        sil = evac.tile([P, 512], f32, tag="sil")
                        nc.scalar.activation(sil[:, :csz], ps1[:, :csz], silu)
                        nc.vector.tensor_mul(
                            gt[fp][:, coff:coff + csz], sil[:, :csz], ps3[:, :csz]
                        )

            # w2 / routing-weight loads are only needed in phase B; emit
            # them after phase A so the DMA queues serve phase A first.
            cvt = []
            for ck in range(CK):
                m = min(P, C - ck * P)
                t = persist.tile([P, 1], f32, tag=f"cv{ck}", name=f"cv{ck}")
                nc.sync.dma_start(out=t[:m, :], in_=cv[ck * P:ck * P + m, :])
                cvt.append(t)
            w2t = []
            for fp in range(FP):
                t = persist.tile([P, H], bf, tag=f"w2_{fp}", name=f"w2_{fp}")
                nc.sync.dma_start(out=t[:], in_=w2p[fp])
                w2t.append(t)

            # Phase B: outT chunk [128 tokens, 1024] = sum_f gT.T @ w2T,
            # scaled by the per-token routing weight on eviction.
            with tc.tile_pool(name="psB", bufs=2, space="PSUM") as psB:
                for ck in range(CK):
                    m = min(P, C - ck * P)
                    pb0 = psB.tile([P, 512], f32, tag="pb0")
                    pb1 = psB.tile([P, 512], f32, tag="pb1")
                    for fp in range(FP):
                        lhs = gt[fp][:, ck * P:ck * P + m]
                        nc.tensor.matmul(pb0[:m, :], lhs, w2t[fp][:, 0:512],
                                         start=(fp == 0), stop=(fp == FP - 1))
                        nc.tensor.matmul(pb1[:m, :], lhs, w2t[fp][:, 512:1024],
                                         start=(fp == 0), stop=(fp == FP - 1))
                    o = ost.tile([P, H], f32, tag="o")
                    nc.scalar.activation(o[:m, 0:512], pb0[:m, :], copy,
                                         scale=cvt[ck][:m, :])
                    nc.scalar.activation(o[:m, 512:1024], pb1[:m, :], copy,
                                         scale=cvt[ck][:m, :])
                    nc.sync.dma_start(out=out[ck * P:ck * P + m, :], in_=o[:m, :])

    nc.compile()
    return nc


def kernel(hidden_states, gate_w, w1, w2, w3, _trace=False):
    global LAST_EXEC_TIME_NS
    _ensure_axon_hooks_stub()
    from concourse.bass_utils import run_bass_kernel_spmd

    x = np.asarray(hidden_states, dtype=np.float32).reshape(-1, H)
    gate_w = np.asarray(gate_w, dtype=np.float32)
    w1 = np.asarray(w1, dtype=np.float32)
    w2 = np.asarray(w2, dtype=np.float32)
    w3 = np.asarray(w3, dtype=np.float32)
    T = x.shape[0]

    # Router (f32, same math as the module): softmax over experts, top-2,
    # renormalized weights.
    logits = x @ gate_w.T
    p = np.exp(logits - logits.max(-1, keepdims=True))
    p /= p.sum(-1, keepdims=True)
    sel = np.argpartition(-p, TOP_K - 1, axis=-1)[:, :TOP_K]
    rw = np.take_along_axis(p, sel, axis=-1)
    rw = rw / rw.sum(-1, keepdims=True)

    idx_e, cv_e = [], []
    for e in range(E):
        hit = sel == e                      # [T, K]
        idx = np.nonzero(hit.any(axis=1))[0]
        w = np.where(hit[idx, 0], rw[idx, 0], rw[idx, 1])
        idx_e.append(idx)
        cv_e.append(w.astype(np.float32))

    cmax = max(len(i) for i in idx_e)
    C = max(cmax, 512)
    # SBUF budget (xe + gT residents) caps C; actual data peaks ~1071.
    assert C <= 1408, f"capacity {C} exceeds SBUF plan"

    if C not in _BUILD_CACHE:
        _BUILD_CACHE[C] = _build(C)
    nc = _BUILD_CACHE[C]

    x_bf = x.astype(BF16)
    in_maps = []
    for e in range(E):
        n = len(idx_e[e])
        xeT = np.zeros((H, C), dtype=BF16)
        xeT[:, :n] = x_bf[idx_e[e]].T
        w1pk = np.ascontiguousarray(
            w1[e].astype(BF16).reshape(FP, P, HK, P).transpose(0, 3, 2, 1)
        ).reshape(FP, P, H)
        w3pk = np.ascontiguousarray(
            w3[e].astype(BF16).reshape(FP, P, HK, P).transpose(0, 3, 2, 1)
        ).reshape(FP, P, H)
        w2pk = np.ascontiguousarray(w2[e].T.astype(BF16)).reshape(FP, P, H)
        cvp = np.zeros((C, 1), dtype=np.float32)
        cvp[:n, 0] = cv_e[e]
        in_maps.append({
            "xe": np.ascontiguousarray(xeT.reshape(HK, P, C)),
            "w1p": w1pk,
            "w3p": w3pk,
            "w2p": w2pk,
            "cv": cvp,
        })

    res = run_bass_kernel_spmd(
        nc, in_maps, core_ids=list(range(N_CORES)), trace=_trace
    )
    LAST_EXEC_TIME_NS = res.exec_time_ns

    out = np.zeros((T, H), dtype=np.float32)
    for e in range(E):
        n = len(idx_e[e])
        oe = res.results[e]["out"].reshape(C, H)[:n]
        out[idx_e[e]] += oe
    return out.reshape(B, S, H)
